# revision 3
# baseline (speedup 1.0000x reference)
"""Trainium2 Bass kernel for multi-head attention (B=4, N=2048, C=1024, H=16).

Sharding (8 cores, no collectives): core c handles batch b = c//2 and
sequence-half h2 = c%2 (q rows [h2*1024, h2*1024+1024)). Each core computes
k/v for the full sequence of its batch (duplicated within the pair), its
q-half, attention for all 16 heads, and the output projection for its rows.
Host concatenates the 8 row-blocks.

Engine layout per core:
  - all matmuls in fp32r (full PE rate, ~1.2e-4 input rounding).
  - scores^T [128k, 1024q] fp32 PSUM chunks -> ACT exp -> PT fp32r SBUF.
  - attn@V via PE with v augmented by a ones column (denominator for free);
    ctx^T accumulated in PSUM [65, 1024].
  - softmax normalize: DVE copy denom -> gpsimd partition_broadcast ->
    DVE reciprocal_approx_fast -> DVE multiply -> ctx^T fp32r -> DRAM.
  - no max-subtraction: scores are ~N(0, 0.17), exp cannot overflow.
"""
import sys

sys.path.insert(0, "/opt/trn_rl_repo")

import numpy as np

B, N, C = 4, 2048, 1024
H = 16
D = C // H
SCALE = np.float32(1.0) / np.sqrt(D).astype(np.float32)
NCORES = 8
NLOC = N // 2            # q rows per core
NKC = N // 128           # 16 k-chunks
NPAIR = H // 2           # 8 head pairs
VW = 65                  # v columns + ones
_cache = {}


def _round_fp32r(a):
    u = np.ascontiguousarray(a, dtype=np.float32).view(np.uint32).astype(np.uint64)
    lsb = (u >> 12) & 1
    u = (u + 0x7FF + lsb) & 0xFFFFF000
    return u.astype(np.uint32).view(np.float32)


def _build():
    import concourse.bacc as bacc
    import concourse.tile as tile
    import concourse.mybir as mybir

    f32 = mybir.dt.float32
    f32r = mybir.dt.float32r

    nc = bacc.Bacc("TRN2", target_bir_lowering=False, debug=False,
                   num_devices=NCORES)

    xT_d = nc.dram_tensor("xT", [C, N], f32r, kind="ExternalInput").ap()
    qxT_d = nc.dram_tensor("qxT", [C, NLOC], f32r, kind="ExternalInput").ap()
    wq_d = nc.dram_tensor("wq", [C, C], f32r, kind="ExternalInput").ap()
    wk_d = nc.dram_tensor("wk", [C, C], f32r, kind="ExternalInput").ap()
    wv_d = nc.dram_tensor("wv", [C, C], f32r, kind="ExternalInput").ap()
    wo_d = nc.dram_tensor("wo", [C, C], f32r, kind="ExternalInput").ap()
    bo_d = nc.dram_tensor("bo_b", [128, C], f32, kind="ExternalInput").ap()
    out_d = nc.dram_tensor("out", [NLOC, C], f32, kind="ExternalOutput").ap()
    # ctx^T staging in DRAM: [cc, c_in_chunk, n]
    ctx_d = nc.dram_tensor("ctx_st", [NPAIR, 128, NLOC], f32r).ap()

    def dma_x(eng, tile_ap, cc, nb, src=None):
        src = xT_d if src is None else src
        eng.dma_start(out=tile_ap,
                      in_=src[cc * 128:(cc + 1) * 128, nb * 512:(nb + 1) * 512])

    with tile.TileContext(nc) as tc:
        with tc.tile_pool(name="mm_ps", bufs=2, space="PSUM") as mm_ps, \
             tc.tile_pool(name="sc_ps", bufs=2, space="PSUM") as sc_ps, \
             tc.tile_pool(name="ctx_ps", bufs=1, space="PSUM") as ctx_ps:
            with tc.tile_pool(name="va_pool", bufs=1) as va_pool, \
                 tc.tile_pool(name="xs_pool", bufs=6) as xs_pool, \
                 tc.tile_pool(name="w_pool", bufs=2) as w_pool, \
                 tc.tile_pool(name="kq_pool", bufs=2) as kq_pool, \
                 tc.tile_pool(name="pt_pool", bufs=2) as pt_pool, \
                 tc.tile_pool(name="nrm_pool", bufs=1) as nrm_pool:
                # ---- V for all heads: VA [128, H * NKC * VW] fp32r ----
                VA = va_pool.tile([128, H * NKC * VW], f32r)
                nc.vector.memset(
                    VA[:].bitcast(f32).rearrange("p (hk w) -> p hk w",
                                                 w=VW)[:, :, 64], 1.0)
                with tc.tile_pool(name="xv_pool", bufs=10) as xv_pool, \
                     tc.tile_pool(name="wv_pool", bufs=1) as wv_pool:
                    for dh in range(2):
                        wv_t = wv_pool.tile([128, 8 * 512], f32r, name="wv")
                        nc.sync.dma_start(
                            out=wv_t[:].rearrange("p (cc d) -> p cc d", cc=8),
                            in_=wv_d[:, dh * 512:(dh + 1) * 512]
                                .rearrange("(cc p) d -> p cc d", p=128))
                        for nb in range(N // 512):
                            xv = [xv_pool.tile([128, 512], f32r, name="xv")
                                  for _ in range(8)]
                            for cc in range(8):
                                dma_x(nc.scalar, xv[cc][:], cc, nb)
                            for sub in range(4):
                                ps = mm_ps.tile([128, 512], f32)
                                for cc in range(8):
                                    nc.tensor.matmul(
                                        ps[:],
                                        lhsT=xv[cc][:, sub * 128:(sub + 1) * 128],
                                        rhs=wv_t[:, cc * 512:(cc + 1) * 512],
                                        start=(cc == 0), stop=(cc == 7))
                                nsub = nb * 4 + sub
                                nc.vector.tensor_copy(
                                    VA[:].rearrange("p (h c) -> p h c", h=H)
                                       [:, dh * 8:(dh + 1) * 8,
                                        nsub * VW:nsub * VW + 64],
                                    ps[:].rearrange("p (h d) -> p h d", h=8))

                # ---- per head-pair: qT/kT production + attention ----
                for hp in range(NPAIR):
                    wk_t = w_pool.tile([128, 8 * 128], f32r, name="wkp")
                    nc.sync.dma_start(
                        out=wk_t[:].rearrange("p (cc d) -> p cc d", cc=8),
                        in_=wk_d[:, hp * 128:(hp + 1) * 128]
                            .rearrange("(cc p) d -> p cc d", p=128))
                    wq_t = w_pool.tile([128, 8 * 128], f32r, name="wqp")
                    nc.sync.dma_start(
                        out=wq_t[:].rearrange("p (cc d) -> p cc d", cc=8),
                        in_=wq_d[:, hp * 128:(hp + 1) * 128]
                            .rearrange("(cc p) d -> p cc d", p=128))

                    kT = kq_pool.tile([128, N], f32r, name="kT")
                    for nb in range(N // 512):
                        ps = mm_ps.tile([128, 512], f32)
                        for cc in range(8):
                            xt = xs_pool.tile([128, 512], f32r, name="xs")
                            dma_x((nc.sync, nc.gpsimd)[cc % 2], xt[:], cc, nb)
                            nc.tensor.matmul(
                                ps[:], lhsT=wk_t[:, cc * 128:(cc + 1) * 128],
                                rhs=xt[:], start=(cc == 0), stop=(cc == 7))
                        nc.vector.tensor_copy(kT[:, nb * 512:(nb + 1) * 512],
                                              ps[:])
                    qT = kq_pool.tile([128, NLOC], f32r, name="qT")
                    for nb in range(NLOC // 512):
                        ps = mm_ps.tile([128, 512], f32)
                        for cc in range(8):
                            xt = xs_pool.tile([128, 512], f32r, name="xs")
                            dma_x((nc.sync, nc.gpsimd)[cc % 2], xt[:], cc, nb,
                                  src=qxT_d)
                            nc.tensor.matmul(
                                ps[:], lhsT=wq_t[:, cc * 128:(cc + 1) * 128],
                                rhs=xt[:], start=(cc == 0), stop=(cc == 7))
                        nc.vector.tensor_copy(qT[:, nb * 512:(nb + 1) * 512],
                                              ps[:])

                    for hh in range(2):
                        h = hp * 2 + hh
                        r0 = hh * 64
                        ctx_p = ctx_ps.tile([VW, NLOC], f32)
                        for kc in range(NKC):
                            sc_p = sc_ps.tile([128, NLOC], f32)
                            for j in range(NLOC // 512):
                                nc.tensor.matmul(
                                    sc_p[:, j * 512:(j + 1) * 512],
                                    lhsT=kT[r0:r0 + 64,
                                            kc * 128:(kc + 1) * 128],
                                    rhs=qT[r0:r0 + 64, j * 512:(j + 1) * 512],
                                    start=True, stop=True)
                            pt = pt_pool.tile([128, NLOC], f32r)
                            nc.scalar.activation(
                                pt[:], sc_p[:], mybir.ActivationFunctionType.Exp)
                            for j in range(NLOC // 512):
                                nc.tensor.matmul(
                                    ctx_p[:, j * 512:(j + 1) * 512],
                                    lhsT=VA[:, (h * NKC + kc) * VW:
                                            (h * NKC + kc + 1) * VW],
                                    rhs=pt[:, j * 512:(j + 1) * 512],
                                    start=(kc == 0), stop=(kc == NKC - 1))
                        den = nrm_pool.tile([1, NLOC], f32, name="den")
                        nc.vector.tensor_copy(den[:], ctx_p[64:65, :])
                        den_b = nrm_pool.tile([64, NLOC], f32, name="den_b")
                        nc.gpsimd.partition_broadcast(den_b[:], den[:])
                        rec = nrm_pool.tile([64, NLOC], f32, name="rec")
                        nc.vector.reciprocal_approx_fast(out=rec[:], in_=den_b[:])
                        cst = nrm_pool.tile([64, NLOC], f32r, name="cst", bufs=2)
                        nc.vector.tensor_tensor(
                            out=cst[:], in0=ctx_p[:64, :], in1=rec[:],
                            op=mybir.AluOpType.mult)
                        nc.gpsimd.dma_start(
                            out=ctx_d[hp, r0:r0 + 64, :], in_=cst[:])

            # ---- projection: out[n, :] = ctx^T.T @ wo + bo ----
            with tc.tile_pool(name="wo_pool", bufs=1) as wo_pool, \
                 tc.tile_pool(name="cl_pool", bufs=10) as cl_pool, \
                 tc.tile_pool(name="out_pool", bufs=3) as out_pool:
                wo_t = wo_pool.tile([128, 8 * C], f32r)
                nc.sync.dma_start(
                    out=wo_t[:].rearrange("p (cc d) -> p cc d", cc=8),
                    in_=wo_d.rearrange("(cc p) d -> p cc d", p=128))
                bo_t = wo_pool.tile([128, C], f32)
                nc.sync.dma_start(out=bo_t[:], in_=bo_d)
                for nt in range(NLOC // 128):
                    cl = [cl_pool.tile([128, 128], f32r, name="cl")
                          for _ in range(8)]
                    for cc in range(8):
                        nc.scalar.dma_start(
                            out=cl[cc][:],
                            in_=ctx_d[cc, :, nt * 128:(nt + 1) * 128])
                    for ch in range(2):
                        ps = mm_ps.tile([128, 512], f32)
                        for cc in range(8):
                            nc.tensor.matmul(
                                ps[:], lhsT=cl[cc][:],
                                rhs=wo_t[:, cc * C + ch * 512:
                                         cc * C + ch * 512 + 512],
                                start=(cc == 0), stop=(cc == 7))
                        ot = out_pool.tile([128, 512], f32)
                        nc.vector.tensor_tensor(
                            out=ot[:], in0=ps[:],
                            in1=bo_t[:, ch * 512:(ch + 1) * 512],
                            op=mybir.AluOpType.add)
                        nc.sync.dma_start(
                            out=out_d[nt * 128:(nt + 1) * 128,
                                      ch * 512:(ch + 1) * 512],
                            in_=ot[:])

    nc.compile()
    return nc


def kernel(x, Wq, Wk, Wv, Wo, bo, _trace=False):
    from concourse.bass_utils import run_bass_kernel_spmd

    if "nc" not in _cache:
        _cache["nc"] = _build()
    nc = _cache["nc"]

    x = np.asarray(x, dtype=np.float32)
    wq = _round_fp32r(np.asarray(Wq, dtype=np.float32) * SCALE)
    wk = _round_fp32r(Wk)
    wv = _round_fp32r(Wv)
    wo = _round_fp32r(Wo)
    bo_b = np.ascontiguousarray(
        np.broadcast_to(np.asarray(bo, dtype=np.float32), (128, C)))

    in_maps = []
    for c in range(NCORES):
        b, h2 = divmod(c, 2)
        xT = _round_fp32r(x[b].T)
        in_maps.append({
            "xT": xT,
            "qxT": np.ascontiguousarray(xT[:, h2 * NLOC:(h2 + 1) * NLOC]),
            "wq": wq, "wk": wk, "wv": wv, "wo": wo, "bo_b": bo_b,
        })

    res = run_bass_kernel_spmd(nc, in_maps, core_ids=list(range(NCORES)),
                               trace=_trace, trace_cores=[0] if _trace else None)
    out = np.empty((B, N, C), dtype=np.float32)
    for c in range(NCORES):
        b, h2 = divmod(c, 2)
        out[b, h2 * NLOC:(h2 + 1) * NLOC, :] = res.results[c]["out"]
    if _trace:
        _cache["last_trace"] = res
    return out


# revision 4
# speedup vs baseline: 1.8376x; 1.8376x over previous
"""Trainium2 Bass kernel for multi-head attention (B=4, N=2048, C=1024, H=16).

Sharding (8 cores, no collectives): core c handles batch b = c//2 and
sequence-half h2 = c%2 (q rows [h2*1024, h2*1024+1024)). Each core computes
k/v for the full sequence of its batch (duplicated within the pair), its
q-half, attention for all 16 heads, and the output projection for its rows.
Host concatenates the 8 row-blocks. Host rotates x columns so the local
q-half is always cols [0, 1024) (k-order permutation is softmax-invariant).

Engine layout per core:
  - all matmul operands fp16 (full PE rate + fast weight load); PSUM fp32.
  - x^T resident in SBUF [1024, 2048] fp16; QKV reads it directly.
  - scores^T [128k, 1024q] fp32 PSUM chunks -> ACT exp -> PT fp16 SBUF.
  - attn@V via PE with v augmented by a ones column (denominator for free);
    ctx^T accumulated in PSUM [65, 1024].
  - softmax normalize: DVE copy denom -> gpsimd partition_broadcast ->
    DVE reciprocal_approx_fast -> DVE multiply -> ctx^T fp16 resident.
  - no max-subtraction: scores are ~N(0, 0.17), exp cannot overflow.
"""
import sys

sys.path.insert(0, "/opt/trn_rl_repo")

import numpy as np

B, N, C = 4, 2048, 1024
H = 16
D = C // H
SCALE = np.float32(1.0) / np.sqrt(D).astype(np.float32)
NCORES = 8
NLOC = N // 2            # q rows per core
NKC = N // 128           # 16 k-chunks
NPAIR = H // 2           # 8 head pairs
VW = 65                  # v columns + ones
_cache = {}


def _build():
    import concourse.bacc as bacc
    import concourse.tile as tile
    import concourse.mybir as mybir

    f32 = mybir.dt.float32
    f16 = mybir.dt.float16

    nc = bacc.Bacc("TRN2", target_bir_lowering=False, debug=False,
                   num_devices=NCORES)

    xT_d = nc.dram_tensor("xT", [C, N], f16, kind="ExternalInput").ap()
    wq_d = nc.dram_tensor("wq", [C, C], f16, kind="ExternalInput").ap()
    wk_d = nc.dram_tensor("wk", [C, C], f16, kind="ExternalInput").ap()
    wv_d = nc.dram_tensor("wv", [C, C], f16, kind="ExternalInput").ap()
    wo_d = nc.dram_tensor("wo", [C, C], f16, kind="ExternalInput").ap()
    bo_d = nc.dram_tensor("bo_b", [128, C], f32, kind="ExternalInput").ap()
    out_d = nc.dram_tensor("out", [NLOC, C], f32, kind="ExternalOutput").ap()

    with tile.TileContext(nc) as tc:
        with tc.tile_pool(name="mm_ps", bufs=2, space="PSUM") as mm_ps, \
             tc.tile_pool(name="sc_ps", bufs=2, space="PSUM") as sc_ps, \
             tc.tile_pool(name="ctx_ps", bufs=1, space="PSUM") as ctx_ps, \
             tc.tile_pool(name="big", bufs=1) as big, \
             tc.tile_pool(name="w_pool", bufs=2) as w_pool, \
             tc.tile_pool(name="kq_pool", bufs=2) as kq_pool, \
             tc.tile_pool(name="pt_pool", bufs=3) as pt_pool, \
             tc.tile_pool(name="nrm_pool", bufs=1) as nrm_pool, \
             tc.tile_pool(name="out_pool", bufs=3) as out_pool:
            # resident tensors
            XT = big.tile([128, 8 * N], f16, name="XT")      # [cc, 128c, n]
            nc.sync.dma_start(
                out=XT[:].rearrange("p (cc n) -> p cc n", cc=8),
                in_=xT_d.rearrange("(cc p) n -> p cc n", p=128))
            VA = big.tile([128, H * NKC * VW], f16, name="VA")
            nc.vector.memset(
                VA[:].bitcast(f16).rearrange("p (hk w) -> p hk w",
                                             w=VW)[:, :, 64], 1.0)
            ctxT = big.tile([128, NPAIR * NLOC], f16, name="ctxT")
            WO = big.tile([128, 8 * C], f16, name="WO")
            nc.sync.dma_start(
                out=WO[:].rearrange("p (cc d) -> p cc d", cc=8),
                in_=wo_d.rearrange("(cc p) d -> p cc d", p=128))
            BO = big.tile([128, C], f32, name="BO")
            nc.sync.dma_start(out=BO[:], in_=bo_d)

            # ---- V for all heads ----
            for dh in range(2):
                wv_t = w_pool.tile([128, 8 * 512], f16, name="wv")
                nc.sync.dma_start(
                    out=wv_t[:].rearrange("p (cc d) -> p cc d", cc=8),
                    in_=wv_d[:, dh * 512:(dh + 1) * 512]
                        .rearrange("(cc p) d -> p cc d", p=128))
                for nsub in range(NKC):
                    ps = mm_ps.tile([128, 512], f32)
                    for cc in range(8):
                        nc.tensor.matmul(
                            ps[:],
                            lhsT=XT[:, cc * N + nsub * 128:
                                    cc * N + (nsub + 1) * 128],
                            rhs=wv_t[:, cc * 512:(cc + 1) * 512],
                            start=(cc == 0), stop=(cc == 7))
                    nc.vector.tensor_copy(
                        VA[:].rearrange("p (h c) -> p h c", h=H)
                           [:, dh * 8:(dh + 1) * 8, nsub * VW:nsub * VW + 64],
                        ps[:].rearrange("p (h d) -> p h d", h=8))

            # ---- per head-pair: qT/kT production + attention ----
            for hp in range(NPAIR):
                wk_t = w_pool.tile([128, 8 * 128], f16, name="wkp")
                nc.sync.dma_start(
                    out=wk_t[:].rearrange("p (cc d) -> p cc d", cc=8),
                    in_=wk_d[:, hp * 128:(hp + 1) * 128]
                        .rearrange("(cc p) d -> p cc d", p=128))
                wq_t = w_pool.tile([128, 8 * 128], f16, name="wqp")
                nc.sync.dma_start(
                    out=wq_t[:].rearrange("p (cc d) -> p cc d", cc=8),
                    in_=wq_d[:, hp * 128:(hp + 1) * 128]
                        .rearrange("(cc p) d -> p cc d", p=128))

                kT = kq_pool.tile([128, N], f16, name="kT")
                for nb in range(N // 512):
                    ps = mm_ps.tile([128, 512], f32)
                    for cc in range(8):
                        nc.tensor.matmul(
                            ps[:], lhsT=wk_t[:, cc * 128:(cc + 1) * 128],
                            rhs=XT[:, cc * N + nb * 512:cc * N + (nb + 1) * 512],
                            start=(cc == 0), stop=(cc == 7))
                    nc.vector.tensor_copy(kT[:, nb * 512:(nb + 1) * 512], ps[:])
                qT = kq_pool.tile([128, NLOC], f16, name="qT")
                for nb in range(NLOC // 512):
                    ps = mm_ps.tile([128, 512], f32)
                    for cc in range(8):
                        nc.tensor.matmul(
                            ps[:], lhsT=wq_t[:, cc * 128:(cc + 1) * 128],
                            rhs=XT[:, cc * N + nb * 512:cc * N + (nb + 1) * 512],
                            start=(cc == 0), stop=(cc == 7))
                    nc.vector.tensor_copy(qT[:, nb * 512:(nb + 1) * 512], ps[:])

                for hh in range(2):
                    h = hp * 2 + hh
                    r0 = hh * 64
                    ctx_p = ctx_ps.tile([VW, NLOC], f32)
                    for kc in range(NKC):
                        sc_p = sc_ps.tile([128, NLOC], f32)
                        for j in range(NLOC // 512):
                            nc.tensor.matmul(
                                sc_p[:, j * 512:(j + 1) * 512],
                                lhsT=kT[r0:r0 + 64, kc * 128:(kc + 1) * 128],
                                rhs=qT[r0:r0 + 64, j * 512:(j + 1) * 512],
                                start=True, stop=True)
                        pt = pt_pool.tile([128, NLOC], f16)
                        nc.scalar.activation(
                            pt[:], sc_p[:], mybir.ActivationFunctionType.Exp)
                        for j in range(NLOC // 512):
                            nc.tensor.matmul(
                                ctx_p[:, j * 512:(j + 1) * 512],
                                lhsT=VA[:, (h * NKC + kc) * VW:
                                        (h * NKC + kc + 1) * VW],
                                rhs=pt[:, j * 512:(j + 1) * 512],
                                start=(kc == 0), stop=(kc == NKC - 1))
                    den = nrm_pool.tile([1, NLOC], f32, name="den")
                    nc.vector.tensor_copy(den[:], ctx_p[64:65, :])
                    den_b = nrm_pool.tile([64, NLOC], f32, name="den_b")
                    nc.gpsimd.partition_broadcast(den_b[:], den[:])
                    rec = nrm_pool.tile([64, NLOC], f32, name="rec")
                    nc.vector.reciprocal_approx_fast(out=rec[:], in_=den_b[:])
                    nc.vector.tensor_tensor(
                        out=ctxT[r0:r0 + 64, hp * NLOC:(hp + 1) * NLOC],
                        in0=ctx_p[:64, :], in1=rec[:],
                        op=mybir.AluOpType.mult)

            # ---- projection ----
            for nt in range(NLOC // 128):
                for ch in range(2):
                    ps = mm_ps.tile([128, 512], f32)
                    for cc in range(8):
                        nc.tensor.matmul(
                            ps[:],
                            lhsT=ctxT[:, cc * NLOC + nt * 128:
                                      cc * NLOC + nt * 128 + 128],
                            rhs=WO[:, cc * C + ch * 512:cc * C + ch * 512 + 512],
                            start=(cc == 0), stop=(cc == 7))
                    ot = out_pool.tile([128, 512], f32)
                    nc.vector.tensor_tensor(
                        out=ot[:], in0=ps[:], in1=BO[:, ch * 512:(ch + 1) * 512],
                        op=mybir.AluOpType.add)
                    nc.gpsimd.dma_start(
                        out=out_d[nt * 128:(nt + 1) * 128,
                                  ch * 512:(ch + 1) * 512],
                        in_=ot[:])

    nc.compile()
    return nc


def kernel(x, Wq, Wk, Wv, Wo, bo, _trace=False):
    from concourse.bass_utils import run_bass_kernel_spmd

    if "nc" not in _cache:
        _cache["nc"] = _build()
    nc = _cache["nc"]

    x = np.asarray(x, dtype=np.float32)
    wq = (np.asarray(Wq, dtype=np.float32) * SCALE).astype(np.float16)
    wk = np.asarray(Wk, dtype=np.float32).astype(np.float16)
    wv = np.asarray(Wv, dtype=np.float32).astype(np.float16)
    wo = np.asarray(Wo, dtype=np.float32).astype(np.float16)
    bo_b = np.ascontiguousarray(
        np.broadcast_to(np.asarray(bo, dtype=np.float32), (128, C)))

    in_maps = []
    for c in range(NCORES):
        b, h2 = divmod(c, 2)
        xT = x[b].T.astype(np.float16)
        # rotate so the local q-half is cols [0, NLOC)
        xT_rot = np.ascontiguousarray(np.roll(xT, -h2 * NLOC, axis=1))
        in_maps.append({"xT": xT_rot, "wq": wq, "wk": wk, "wv": wv,
                        "wo": wo, "bo_b": bo_b})

    res = run_bass_kernel_spmd(nc, in_maps, core_ids=list(range(NCORES)),
                               trace=_trace, trace_cores=[0] if _trace else None)
    out = np.empty((B, N, C), dtype=np.float32)
    for c in range(NCORES):
        b, h2 = divmod(c, 2)
        out[b, h2 * NLOC:(h2 + 1) * NLOC, :] = res.results[c]["out"]
    if _trace:
        _cache["last_trace"] = res
    return out
